# revision 46
# baseline (speedup 1.0000x reference)
"""Trainium2 Bass kernel for Llama SmartKV decode attention (GQA, q_len=1).

Sharding: tensor-parallel over KV heads — core c owns kv head c and its
GQA group of 4 query heads (slices of Wq/Wk/Wv/Wo), plus that head's
quantized KV cache. Each core computes its partial o_proj output; the
host sums the 8 partials (the all-reduce).

Host-side input prep (pure numpy, done once per call):
  - RoPE (cos/sin) and the 1/sqrt(D) score scale are folded into Wq/Wk.
  - K/V stay int8 codes on device; scales ride separately (fp32).
  - softmax exp uses a constant bias which cancels in the normalization
    but keeps exp() in fp16 range for any input scale.

Schedule (v3): the wire runs serial full-rate streams in consumption
order — V int8 (plain DMA), wqkv fp16, K (int8->fp16 cast DMA), then
wo in 8 column blocks consumed by a streaming o_proj.  V is upconverted
int8->fp16 on the otherwise-idle vector engine, which halves V's DMA
SBUF-write bytes.  Engine roles: PE matmuls; DVE V-upconvert + score
scaling + reciprocal; ACT exp/copies; GPSIMD pprime scaling +
denominator partials.  Every instruction carries at most ONE semaphore
wait (hardware limit) — cross-engine chains go through touch ops and
same-queue DMA ordering.
"""

import os

os.environ.setdefault("BY_DEFAULT_DISABLE_SUBTILE_DEPS", "1")

import numpy as np

import concourse.bass as bass
import concourse.mybir as mybir
import concourse.tile as tile
from concourse.bass_utils import run_bass_kernel_spmd
from concourse.tile_rust import add_dep_helper

H, HKV, D, HID, S = 32, 8, 128, 4096, 32768
G = H // HKV  # 4 query heads per core
NCORES = 8
KC = HID // 128  # 32 contraction chunks for projections
NCH = S // 128  # 256 score/PV chunks of 128 tokens
KTILE = 4096  # tokens per K-cache DMA tile
VCH = 32  # s-chunks per V fp16 tile
WJ = 4  # projection j-chunks per wqkv DMA tile
GRP = 32  # score chunks per softmax group (8 groups, one V tile each)
NGRP = NCH // GRP
F16 = mybir.dt.float16
I8 = mybir.dt.int8
F32 = mybir.dt.float32
EXP_BIAS = -11.0  # exp(s + B): cancels in softmax, keeps fp16 in range

_CACHE = {}


def _reduce_dma_waits(nc):
    """Drop transitively-implied waits (vector-clock walk over the
    scheduled program).  TRN2 instructions have a single HW wait slot;
    Tile's sem assignment is not transitively minimal, so redundant
    waits implied through engine FIFOs or DMA-queue ordering are
    deleted.  Raises if a DMA still needs more than one wait."""
    import bass_rust as _br

    insts = []
    for f in nc.m.functions:
        for bb in f.blocks:
            insts.extend(bb.instructions)

    cum = {}  # sem name -> cumulative value so far in schedule order
    snaps = {}  # sem name -> list of (cumval, knowledge dict)
    streams = {}  # stream key -> knowledge dict (sem name -> value known >=)

    def know_at(sem, val):
        # knowledge of the producer that first brought `sem` to >= val
        for cv, kn in snaps.get(sem, ()):
            if cv >= val:
                return kn
        return None

    for inst in insts:
        si = inst.sync_info
        if si is None:
            continue
        waits = list(si.on_wait)
        ups = list(si.on_update)
        if ups and ups[0].ant_name.startswith(("DMASW", "DMAHW")):
            skey = ups[0].ant_name
        else:
            skey = f"eng:{inst.engine}"
        kn = dict(streams.get(skey, ()))

        imm = [
            w
            for w in waits
            if w.wait_mode == "sem-ge-imm" and w.sync_type == "semaphore"
        ]
        if len(imm) == len(waits) > 1:
            keep = []
            for w in waits:
                others = dict(kn)
                for w2 in waits:
                    if w2 is w:
                        continue
                    others[w2.ant_name] = max(
                        others.get(w2.ant_name, 0), w2.wait_value
                    )
                    k2 = know_at(w2.ant_name, w2.wait_value)
                    if k2:
                        for s, v in k2.items():
                            others[s] = max(others.get(s, 0), v)
                if others.get(w.ant_name, 0) >= w.wait_value:
                    continue  # implied: drop
                keep.append(w)
            if len(keep) < len(waits):
                inst.sync_info = _br.SyncInfo(on_wait=keep, on_update=ups)
                waits = keep

        # fold wait knowledge into this instruction's stream knowledge
        for w in waits:
            if w.wait_mode != "sem-ge-imm" or w.sync_type != "semaphore":
                continue
            kn[w.ant_name] = max(kn.get(w.ant_name, 0), w.wait_value)
            k2 = know_at(w.ant_name, w.wait_value)
            if k2:
                for s, v in k2.items():
                    kn[s] = max(kn.get(s, 0), v)
        for u in ups:
            if u.sync_type != "semaphore":
                continue
            cum[u.ant_name] = cum.get(u.ant_name, 0) + u.update_value
            kn[u.ant_name] = max(kn.get(u.ant_name, 0), cum[u.ant_name])
            snaps.setdefault(u.ant_name, []).append((cum[u.ant_name], kn))
        streams[skey] = kn

    bad = [
        (i.name, type(i).__name__, [(w.ant_name, w.wait_value) for w in i.sync_info.on_wait])
        for i in insts
        if i.sync_info is not None
        and len(i.sync_info.on_wait) > 1
        and "DMA" in type(i).__name__.upper()
    ]
    if bad:
        raise RuntimeError(f"instructions still multi-wait: {bad}")


def _bc_g(ap, g=G):
    """Broadcast a [128, C] AP over a trailing g axis -> [128, C, g]."""
    return bass.AP(tensor=ap.tensor, offset=ap.offset, ap=[*ap.ap, [0, g]])


def _build_nc(debug=False):
    nc = bass.Bass()
    hsT = nc.declare_dram_parameter("hsT", [128, KC], F16, isOutput=False)
    wqkv = nc.declare_dram_parameter("wqkv", [KC // WJ, 128, WJ * 768], F16, isOutput=False)
    kT = nc.declare_dram_parameter("kT", [128, S], I8, isOutput=False)
    kvsc = nc.declare_dram_parameter("kvsc", [128, 2 * (S // 128)], F32, isOutput=False)
    v8i = nc.declare_dram_parameter("v8i", [2, 128, S // 2], I8, isOutput=False)
    wo = nc.declare_dram_parameter("wo", [HID // 512, 128, G * 512], F16, isOutput=False)
    out = nc.declare_dram_parameter("out", [1, HID], F16, isOutput=True)

    PS = bass.MemorySpace.PSUM
    with tile.TileContext(nc) as tc:
        with (
            tc.tile_pool(name="const", bufs=1) as cpool,
            tc.tile_pool(name="wqkvp", bufs=4) as wqkv_pool,
            tc.tile_pool(name="kp", bufs=5) as k_pool,
            tc.tile_pool(name="vfp", bufs=8) as vf_pool,
            tc.tile_pool(name="wop", bufs=8) as wo_pool,
            tc.tile_pool(name="sm", bufs=1) as sm,
        ):
            # V-int8 staging pool is scoped: closed right after the
            # upconverts so its SBUF region is reused by the softmax tiles
            # (whose writers chain through DVE, keeping the WAR free)
            v8_ctx = tc.tile_pool(name="v8p", bufs=2)
            v8_pool = v8_ctx.__enter__()
            # ---- Q1 (sync/HWDGE) stream: hsT, V-int8, wqkv (FIFO) ----
            hs_sb = cpool.tile([128, KC], F16)
            nc.sync.dma_start(out=hs_sb, in_=hsT[:, :])
            v8_sb = []
            for t in range(2):
                v8t = v8_pool.tile([128, S // 2], I8, tag="v8")
                nc.sync.dma_start(out=v8t, in_=v8i[t])
                v8_sb.append(v8t)

            # ---- Q0 (gpsimd/SWDGE): merged scales ----
            kvsc_sb = cpool.tile([128, 2 * (S // 128)], F32)
            nc.gpsimd.dma_start(out=kvsc_sb, in_=kvsc[:, :])
            ksc_sb = kvsc_sb[:, 0 : S // 128]
            vsc_sb = kvsc_sb[:, S // 128 : 2 * (S // 128)]

            # constants: ebias on DVE (implied by later DVE waits);
            # ones/ones_row on GPSIMD (implied by later GPS waits)
            ebias_sb = cpool.tile([128, 1], F32)
            nc.vector.memset(ebias_sb, EXP_BIAS)
            ones_sb = cpool.tile([128, 1], F32)
            nc.vector.memset(ones_sb, 1.0)
            ones_row = cpool.tile([1, 128], F32)
            nc.vector.memset(ones_row, 1.0 / 16384.0)

            # tiny touches so later ops inherit the scale-DMA wait through
            # their own engine stream (single-wait discipline); separate
            # tiles per use — subtile dep tracking is disabled
            touch_k = sm.tile([1, 1], F32)
            touch_v = sm.tile([1, 1], F32)
            touch_c = sm.tile([1, 1], F32)
            touch_w = sm.tile([1, 1], F32)
            nc.vector.tensor_copy(out=touch_k, in_=ksc_sb[0:1, 0:1])
            nc.gpsimd.tensor_copy(out=touch_v, in_=vsc_sb[0:1, 0:1])

            # ---- DVE: upconvert V int8 -> fp16, 8 tiles of 32 chunks ----
            v_sb = []
            for t in range(8):
                vt = vf_pool.tile([128, VCH * D], F16, tag="vf")
                src = v8_sb[t // 4][:, (t % 4) * (VCH * D) : (t % 4 + 1) * (VCH * D)]
                nc.vector.tensor_copy(out=vt, in_=src)
                v_sb.append(vt)
            v8_ctx.__exit__(None, None, None)

            # softmax tiles live in the released V-int8 region
            sm2_ctx = tc.tile_pool(name="sm2", bufs=1)
            sm2 = sm2_ctx.__enter__()

            qk_sb = sm.tile([128, G + 1], F16)
            vrow_sb = sm.tile([1, D], F32)

            # ---- q/k/v projections (RoPE+scale pre-folded into weights) ----
            with tc.tile_pool(name="psqk", bufs=1, space=PS) as psqk_pool:
                ps_qk = [
                    psqk_pool.tile([128, 1], F32, name=f"ps_qk{h}", tag=f"qk{h}")
                    for h in range(G + 1)
                ]
                ps_v = psqk_pool.tile([1, D], F32, tag="psv")
                w_dmas = []
                for jj in range(KC // WJ):
                    w_sb = wqkv_pool.tile([128, WJ * 768], F16, tag="wt")
                    w_dmas.append(nc.sync.dma_start(out=w_sb, in_=wqkv[jj]))
                    for c in range(WJ):
                        j = jj * WJ + c
                        for h in range(G + 1):
                            nc.tensor.matmul(
                                ps_qk[h][:, :],
                                lhsT=w_sb[:, c * 768 + h * 128 : c * 768 + (h + 1) * 128],
                                rhs=hs_sb[:, j : j + 1],
                                start=(j == 0),
                                stop=(j == KC - 1),
                            )
                        nc.tensor.matmul(
                            ps_v[:, :],
                            lhsT=hs_sb[:, j : j + 1],
                            rhs=w_sb[:, c * 768 + 640 : c * 768 + 768],
                            start=(j == 0),
                            stop=(j == KC - 1),
                        )
                for h in range(G + 1):
                    nc.vector.tensor_copy(out=qk_sb[:, h : h + 1], in_=ps_qk[h])
                # scaled 2^14 to match the v_scale-folded PV accumulation
                nc.scalar.mul(out=vrow_sb, in_=ps_v, mul=16384.0)

            # ---- Q0 (gpsimd/SWDGE): all four K cast-DMAs upfront, so no
            # later GPS op can head-of-line block the K stream ----
            k_sbs = []
            k_dmas = []
            for co in range(S // KTILE):
                k_sb = k_pool.tile([128, KTILE], F16, tag="kt")
                kd = nc.gpsimd.dma_start(
                    out=k_sb, in_=kT[:, co * KTILE : (co + 1) * KTILE]
                )
                if co == 0:
                    # K stream starts as the wqkv stream drains
                    add_dep_helper(kd.ins, w_dmas[5].ins, sync=True,
                                   reason="K stream as wqkv drains")
                k_sbs.append(k_sb)
                k_dmas.append(kd)

            with (
                tc.tile_pool(name="pssc", bufs=1, space=PS) as pssc_pool,
                tc.tile_pool(name="pspv", bufs=1, space=PS) as pspv_pool,
                tc.tile_pool(name="psms", bufs=1, space=PS) as psms_pool,
            ):
                # ---- current-token score row: s_curT[1, g] = k_cur . q_g ----
                ps_scur = psms_pool.tile([1, G], F32, tag="ms")
                nc.tensor.matmul(
                    ps_scur[:, :], lhsT=qk_sb[:, G : G + 1], rhs=qk_sb[:, 0:G],
                    start=True, stop=True,
                )

                # ---- scores + softmax + PV, interleaved per K tile ----
                # The PE runs strictly in emission order, so PV matmuls for
                # tile n-1 are emitted right after the score matmuls of
                # tile n: the PE chews PV while the next K tile streams in,
                # instead of head-of-line blocking on the K DMA.
                ps_sc = [
                    pssc_pool.tile([128, 512], F32, name=f"ps_sc{b}", tag=f"sc{b}")
                    for b in range(2)
                ]
                probs_sb = [
                    sm2.tile([128, GRP * G], F16, name=f"probs{g}", tag=f"pr{g}")
                    for g in range(NGRP)
                ]
                pprime_sb = [
                    sm2.tile([128, GRP * G], F16, name=f"pprime{g}", tag=f"pp{g}")
                    for g in range(NGRP)
                ]
                scraw_sb = [
                    sm2.tile([128, GRP * G], F32, name=f"scraw{g}", tag=f"sr{g}")
                    for g in range(NGRP)
                ]
                dpart_sb = sm2.tile([128, NGRP * G], F32)
                pcurf_sb = sm.tile([1, G], F32)
                ps_pv = pspv_pool.tile([128, G], F32)
                gpt = KTILE // (GRP * 128)  # softmax groups per K tile

                def emit_softmax(gi):
                    b = (gi * GRP) // 128
                    col = ((gi * GRP) % 128) * 4
                    ksl = ksc_sb[:, gi * GRP : (gi + 1) * GRP]
                    nc.vector.tensor_mul(
                        out=scraw_sb[gi].rearrange("p (c g) -> p c g", g=G),
                        in0=ps_sc[b][:, col : col + GRP * G].rearrange(
                            "p (c g) -> p c g", g=G),
                        in1=_bc_g(ksl),
                    )
                    nc.scalar.activation(
                        out=probs_sb[gi], in_=scraw_sb[gi],
                        func=mybir.ActivationFunctionType.Exp, bias=ebias_sb,
                    )
                    # chain the V-upconvert (DVE) into the GPS stream BEFORE
                    # the pprime mul, so the PV matmuls of this group need
                    # only the single GPS(pprime) wait
                    nc.gpsimd.tensor_copy(out=touch_c, in_=v_sb[gi][0:1, 0:1])
                    vsl = vsc_sb[:, gi * GRP : (gi + 1) * GRP]
                    nc.gpsimd.tensor_mul(
                        out=pprime_sb[gi].rearrange("p (c g) -> p c g", g=G),
                        in0=probs_sb[gi].rearrange("p (c g) -> p c g", g=G),
                        in1=_bc_g(vsl),
                    )
                    nc.vector.reduce_sum(
                        out=dpart_sb[:, gi * G : (gi + 1) * G],
                        in_=probs_sb[gi].rearrange("p (c g) -> p g c", g=G),
                        axis=mybir.AxisListType.X,
                    )

                def emit_pv(gi):
                    for ci in range(GRP):
                        ch = gi * GRP + ci
                        nc.tensor.matmul(
                            ps_pv[:, :],
                            lhsT=v_sb[gi][:, ci * D : (ci + 1) * D],
                            rhs=pprime_sb[gi][:, ci * G : (ci + 1) * G],
                            start=(ch == 0),
                            stop=False,
                        )

                kcpt = KTILE // 128  # score chunks per K tile
                for co in range(S // KTILE):
                    k_sb = k_sbs[co]
                    for ci in range(kcpt):
                        ch = co * kcpt + ci
                        b, col = ch // 128, (ch % 128) * 4
                        nc.tensor.matmul(
                            ps_sc[b][:, col : col + 4],
                            lhsT=k_sb[:, ci * 128 : (ci + 1) * 128],
                            rhs=qk_sb[:, 0:G],
                            start=True,
                            stop=True,
                        )
                    for g in range(gpt):
                        emit_softmax(co * gpt + g)
                    if co == 0:
                        # current-token exp early (after exp-g0 so the ebias
                        # DVE wait is inherited through the ACT stream)
                        nc.scalar.activation(
                            out=pcurf_sb, in_=ps_scur,
                            func=mybir.ActivationFunctionType.Exp,
                            bias=ebias_sb[:1],
                        )
                    if co > 0:
                        for g in range(gpt):
                            emit_pv((co - 1) * gpt + g)
                for g in range(gpt):
                    emit_pv((S // KTILE - 1) * gpt + g)
                nc.tensor.matmul(
                    ps_pv[:, :], lhsT=vrow_sb, rhs=pcurf_sb, start=False, stop=True,
                )

                # ---- denominator and normalization ----
                ps_den = psms_pool.tile([1, NGRP * G], F32, tag="ms")
                nc.tensor.matmul(
                    ps_den[:, :], lhsT=ones_sb, rhs=dpart_sb, start=True, stop=True,
                )
                den_sb = sm.tile([1, NGRP * G], F32)
                nc.scalar.copy(out=den_sb, in_=ps_den)
                d4_sb = sm.tile([1, 4 * G], F32)
                nc.gpsimd.tensor_add(
                    out=d4_sb, in0=den_sb[:, 0 : 4 * G], in1=den_sb[:, 4 * G : 8 * G]
                )
                d2_sb = sm.tile([1, 2 * G], F32)
                nc.gpsimd.tensor_add(
                    out=d2_sb, in0=d4_sb[:, 0 : 2 * G], in1=d4_sb[:, 2 * G : 4 * G]
                )
                dtot_sb = sm.tile([1, G], F32)
                nc.gpsimd.tensor_add(
                    out=dtot_sb, in0=d2_sb[:, 0:G], in1=d2_sb[:, G : 2 * G]
                )
                nc.gpsimd.tensor_add(out=dtot_sb, in0=dtot_sb, in1=pcurf_sb)
                rden_sb = sm.tile([1, G], F32)
                nc.vector.reciprocal(out=rden_sb, in_=dtot_sb)
                # broadcast rden across partitions on PE (ones outer product)
                ps_bc = psms_pool.tile([128, G], F32, tag="ms")
                nc.tensor.matmul(
                    ps_bc[:, :], lhsT=ones_row, rhs=rden_sb, start=True, stop=True
                )
                bc_sb = sm.tile([128, G], F32)
                nc.scalar.copy(out=bc_sb, in_=ps_bc)
                # normalize per head on ACT (per-partition scale = bc column)
                outn_sb = sm.tile([128, G], F16)
                for g in range(G):
                    nc.scalar.activation(
                        out=outn_sb[:, g : g + 1], in_=ps_pv[:, g : g + 1],
                        func=mybir.ActivationFunctionType.Copy,
                        scale=bc_sb[:, g : g + 1],
                    )

            # ---- o_proj: wo streams in last; o_proj consumes each block
            # as it lands.  8 PSUM banks: no WAR waits anywhere. ----
            with tc.tile_pool(name="pso", bufs=8, space=PS) as pso_pool:
                ofin_sb = sm2.tile([1, HID], F16)
                for n in range(HID // 512):
                    wo_sb = wo_pool.tile([128, G * 512], F16, tag="wot")
                    wd = nc.sync.dma_start(out=wo_sb, in_=wo[n])
                    if n == 0:
                        add_dep_helper(wd.ins, k_dmas[5].ins, sync=True,
                                       reason="wo stream overlaps the K tail")
                        # ACT touch: one wait that covers outn + wo block 0
                        tw = nc.scalar.copy(out=touch_w, in_=wo_sb[0:1, 0:1])
                    ps_on = pso_pool.tile([1, 512], F32, tag="on")
                    for g in range(G):
                        mm = nc.tensor.matmul(
                            ps_on[:, :],
                            lhsT=outn_sb[:, g : g + 1],
                            rhs=wo_sb[:, g * 512 : (g + 1) * 512],
                            start=(g == 0),
                            stop=(g == G - 1),
                        )
                        if n == 0 and g == 0:
                            add_dep_helper(getattr(mm, "ins", mm),
                                           getattr(tw, "ins", tw), sync=True,
                                           reason="o_proj gate via ACT touch")
                    nc.scalar.copy(out=ofin_sb[:, n * 512 : (n + 1) * 512], in_=ps_on)
            nc.sync.dma_start(out=out[:, :], in_=ofin_sb)
            sm2_ctx.__exit__(None, None, None)

    _reduce_dma_waits(nc)
    return nc


def _rope_fold(W, nheads, cos, sin, scale=1.0):
    """Fold RoPE rotation (and an optional scalar) into projection weights."""
    W = W.reshape(HID, nheads, D).astype(np.float32)
    half = D // 2
    Wr = np.empty_like(W)
    Wr[:, :, :half] = cos[:half] * W[:, :, :half] - sin[:half] * W[:, :, half:]
    Wr[:, :, half:] = cos[half:] * W[:, :, half:] + sin[half:] * W[:, :, :half]
    return (Wr * np.float32(scale)).reshape(HID, nheads * D)


def _prep_inputs(hidden_states, k_qx, k_scale, v_qx, v_scale, cos, sin, Wq, Wk, Wv, Wo):
    f16 = np.float16
    hs = np.ascontiguousarray(hidden_states.reshape(HID)).astype(np.float32)
    cos = cos.astype(np.float32)
    sin = sin.astype(np.float32)
    Wq_f = _rope_fold(Wq, H, cos, sin, 1.0 / np.sqrt(D))
    Wk_f = _rope_fold(Wk, HKV, cos, sin)
    hsT = np.ascontiguousarray(hs.reshape(KC, 128).T).astype(f16)

    in_maps = []
    for c in range(NCORES):
        qcols = slice(G * c * D, G * (c + 1) * D)
        kvcols = slice(c * D, (c + 1) * D)
        wqkv = np.ascontiguousarray(
            np.concatenate([Wq_f[:, qcols], Wk_f[:, kvcols], Wv[:, kvcols]], axis=1)
            .astype(f16)
            .reshape(KC // 4, 4, 128, 768)
            .transpose(0, 2, 1, 3)
        ).reshape(KC // 4, 128, 4 * 768)
        kT = np.ascontiguousarray(k_qx[:, c, :].astype(np.int8).T)
        # V int8 codes: [tile, 128-token-partition, 4x(32-chunk x 128-d)]
        v8i = np.ascontiguousarray(
            v_qx[:, c, :].astype(np.int8)
            .reshape(2, 4 * VCH, 128, D)
            .transpose(0, 2, 1, 3)
        ).reshape(2, 128, S // 2)
        ksc = np.ascontiguousarray(k_scale[:, c, 0].astype(np.float32).reshape(S // 128, 128).T)
        vsc = np.ascontiguousarray(v_scale[:, c, 0].astype(np.float32).reshape(S // 128, 128).T) * np.float32(16384.0)
        kvsc = np.ascontiguousarray(np.concatenate([ksc, vsc], axis=1))
        # wo block n: [128, g*512+j] = Wo[gcD + g*128 + p, n*512 + j]
        wo = np.ascontiguousarray(
            Wo[G * c * D : G * (c + 1) * D, :].astype(f16)
            .reshape(G, 128, HID // 512, 512)
            .transpose(2, 1, 0, 3)
        ).reshape(HID // 512, 128, G * 512)
        in_maps.append(
            {"hsT": hsT, "wqkv": wqkv, "kT": kT, "v8i": v8i, "wo": wo,
             "kvsc": kvsc}
        )
    return in_maps


def _run(in_maps, trace=False, **kw):
    if "nc" not in _CACHE:
        _CACHE["nc"] = _build_nc()
    return run_bass_kernel_spmd(
        _CACHE["nc"], in_maps, core_ids=list(range(NCORES)), trace=trace, **kw
    )


def kernel(hidden_states, k_qx, k_scale, v_qx, v_scale, cos, sin, Wq, Wk, Wv, Wo):
    in_maps = _prep_inputs(
        hidden_states, k_qx, k_scale, v_qx, v_scale, cos, sin, Wq, Wk, Wv, Wo
    )
    res = _run(in_maps)
    out = np.zeros((1, 1, HID), np.float32)
    for r in res.results:
        out += r["out"].astype(np.float32).reshape(1, 1, HID)
    return out
